# revision 7
# baseline (speedup 1.0000x reference)
"""Type-2 NUFFT (image -> non-uniform k-space) on 8 Trainium2 NeuronCores.

kspace[b,m] = sum_{x,y} image[b,x,y] * exp(-i*(kx_m*(x-128) + ky_m*(y-128)))

Per core (M sharded 8 ways -> 2048 points):
  stage 1 (PE, fp32): A*[m,y+] = sum_x img_fold[x,y+] * trig_x[m,x] where the
    image is folded even/odd along y (y' = y-128; pairs +-j combined), so the
    y-contraction width drops to ~128 per term.
  stage 2 (DVE fused mul+reduce): Re[m] = sum(AB_re * W[0:257]),
    -Im[m] = sum(AB_im * W[128:385]) with one shared trig table
    W = [-Sy' | Cy | Sy'] and psum slots AB_re=[B_odd|A_even],
    AB_im=[B_even|A_odd].

Trig tables on-chip: P = k*grid/(2pi); f = P - round(P) via the fp32
magic-constant trick; sin = Sin(2pi f) on ScalarE (LUT valid on [-pi,pi]);
cos = 1 - 2*Sin(pi f)^2.
"""

import sys

if '/opt/trn_rl_repo' not in sys.path:
    sys.path.insert(0, '/opt/trn_rl_repo')

import numpy as np

B, NX, NY, M, NCORES = 2, 256, 256, 16384, 8
ML = M // NCORES            # 2048 m-points per core
NT = ML // 128              # 16 m-tiles per core
TWO_PI = float(2.0 * np.pi)
PI = float(np.pi)
MAGIC = 12582912.0          # 1.5 * 2**23: (x + MAGIC) - MAGIC == round(x) fp32
NE = NX // 2 + 1            # 129 even-fold cols (y+ = 0..128)
NO = NX // 2                # 128 odd-fold cols  (y+ = 1..127, then y'=-128)
NW = NO + NE + NO           # 385: [-Sy' | Cy | Sy']

_CACHE = {}


def _consts():
    # x-grid row (for PE outer products): (x-128)/(2pi), 256 wide
    xs = (np.arange(NX, dtype=np.float64) - NX // 2) / (2.0 * np.pi)
    xs2pi = xs.astype(np.float32).reshape(1, NX)
    # folded y-grid args, replicated across partitions:
    #   cols 0:128   -> ys_s = [1..127, -128]      (S segment args)
    #   cols 128:257 -> yc   = [0..128]            (C segment args)
    ys_s = np.concatenate([np.arange(1, 128), [-128.0]]).astype(np.float64)
    yc = np.arange(0, 129, dtype=np.float64)
    yargs = (np.concatenate([ys_s, yc]) / (2.0 * np.pi)).astype(np.float32)
    ysb = np.broadcast_to(yargs.reshape(1, NO + NE), (128, NO + NE)).copy()
    ident16 = np.eye(16, dtype=np.float32)
    return xs2pi, ysb, ident16


def _build():
    import concourse.bacc as bacc
    import concourse.mybir as mybir
    from concourse.tile import TileContext

    from concourse.tile_rust import add_dep_helper

    A = mybir.AluOpType
    F = mybir.ActivationFunctionType
    f32 = mybir.dt.float32

    nc = bacc.Bacc("TRN2", target_bir_lowering=False, debug=False)

    image = nc.dram_tensor("image", [B, NX, NY], f32, kind="ExternalInput")
    traj = nc.dram_tensor("traj", [2, ML], f32, kind="ExternalInput")
    xs2pi = nc.dram_tensor("xs2pi", [1, NX], f32, kind="ExternalInput")
    ysb = nc.dram_tensor("ysb", [128, NO + NE], f32, kind="ExternalInput")
    ident16 = nc.dram_tensor("ident16", [16, 16], f32, kind="ExternalInput")
    out = nc.dram_tensor("out", [128, 4 * NT], f32, kind="ExternalOutput")

    with TileContext(nc) as tc:
        with tc.tile_pool(name="const", bufs=1) as cpool, \
             tc.tile_pool(name="xtab", bufs=1) as xpool, \
             tc.tile_pool(name="xscratch", bufs=2) as xs_pool, \
             tc.tile_pool(name="ytab", bufs=3) as ypool, \
             tc.tile_pool(name="work", bufs=3) as wpool:

            # ---------------- constants / inputs ----------------
            xs_sb = cpool.tile([1, NX], f32)
            nc.sync.dma_start(xs_sb[:, :], xs2pi[:, :])
            ysb_sb = cpool.tile([128, NO + NE], f32)
            nc.sync.dma_start(ysb_sb[:, :], ysb[:, :])
            kx_row = cpool.tile([1, ML], f32)
            nc.sync.dma_start(kx_row[:, :], traj[0:1, :])
            id16 = cpool.tile([16, 16], f32)
            nc.sync.dma_start(id16[:, :], ident16[:, :])

            # ky one-per-partition via cheap [16,128] DMA + PE transpose
            ky16 = cpool.tile([16, 128], f32)
            nc.sync.dma_start(
                ky16[:, :], traj[1:2, :].rearrange("o (t p) -> (o t) p", p=128))
            ky_col = cpool.tile([128, NT], f32)

            # image load + even/odd y-fold (on chip)
            img_even = {}
            img_odd = {}
            for b in range(B):
                for k in range(2):
                    raw = wpool.tile([128, NY], f32, tag="imgraw")
                    nc.sync.dma_start(
                        raw[:, :], image[b, k * 128:(k + 1) * 128, :])
                    ev = cpool.tile([128, NE], f32, name=f"ie_{b}_{k}")
                    od = cpool.tile([128, NO], f32, name=f"io_{b}_{k}")
                    # ev[:,0]=img[:,128]; ev[:,1:128]=img[:,129:]+img[:,127:0:-1]
                    # ev[:,128]=img[:,0]; od[:,0:127]=img[:,129:]-img[:,127:0:-1]
                    # od[:,127]=img[:,0]
                    nc.scalar.copy(ev[:, 0:1], raw[:, 128:129])
                    nc.vector.tensor_add(
                        ev[:, 1:128], raw[:, 129:256], raw[:, 127:0:-1])
                    nc.scalar.copy(ev[:, 128:129], raw[:, 0:1])
                    nc.vector.tensor_sub(
                        od[:, 0:127], raw[:, 129:256], raw[:, 127:0:-1])
                    nc.scalar.copy(od[:, 127:128], raw[:, 0:1])
                    img_even[(b, k)] = ev
                    img_odd[(b, k)] = od

            # ---------------- x tables: CxT/SxT [x(2x128), m(2048)] --------
            cxT = [xpool.tile([128, ML], f32, name=f"cxT{h}") for h in range(2)]
            sxT = [xpool.tile([128, ML], f32, name=f"sxT{h}") for h in range(2)]
            with tc.tile_pool(name="psP", bufs=1, space="PSUM") as psP:
                ky_ps = psP.tile([128, 16], f32, tag="kyT")
                nc.tensor.transpose(ky_ps[:, :], ky16[:, :], id16[:, :])
                nc.scalar.copy(ky_col[:, :], ky_ps[:, :])
                for h in range(2):
                    P = psP.tile([128, ML], f32, tag="Px")
                    for j in range(ML // 512):
                        nc.tensor.matmul(
                            P[:, j * 512:(j + 1) * 512],
                            xs_sb[:, h * 128:(h + 1) * 128],
                            kx_row[:, j * 512:(j + 1) * 512],
                            start=True, stop=True)
                    rs = xs_pool.tile([128, ML], f32, tag="xrs")
                    nc.vector.tensor_scalar(
                        rs[:, :], P[:, :], scalar1=MAGIC, scalar2=MAGIC,
                        op0=A.add, op1=A.subtract)
                    fs = xs_pool.tile([128, ML], f32, tag="xfs")
                    nc.vector.scalar_tensor_tensor(
                        fs[:, :], P[:, :], 1.0, rs[:, :],
                        op0=A.mult, op1=A.subtract)
                    nc.scalar.activation(sxT[h][:, :], fs[:, :], F.Sin, scale=TWO_PI)
                    sh = xs_pool.tile([128, ML], f32, tag="xsh")
                    nc.scalar.activation(sh[:, :], fs[:, :], F.Sin, scale=PI)
                    sq = xs_pool.tile([128, ML], f32, tag="xsq")
                    nc.scalar.activation(sq[:, :], sh[:, :], F.Square)
                    nc.vector.tensor_scalar(
                        cxT[h][:, :], sq[:, :], scalar1=-2.0, scalar2=1.0,
                        op0=A.mult, op1=A.add)

            # ---------------- per m-tile main loop ----------------
            out_sb = cpool.tile([128, 4 * NT], f32)
            psAB_cm = tc.tile_pool(name="psAB", bufs=3, space="PSUM")
            psAB = psAB_cm.__enter__()
            for t in range(NT):
                # --- shared y table W = [-Sy'(128) | Cy(129) | Sy'(128)] ---
                u = ky_col[:, t:t + 1]
                p_y = ypool.tile([128, NO + NE], f32, tag="py")
                nc.vector.tensor_scalar(
                    p_y[:, :], ysb_sb[:, :], scalar1=u, scalar2=None, op0=A.mult)
                rs_y = ypool.tile([128, NO + NE], f32, tag="yrs")
                nc.vector.tensor_scalar(
                    rs_y[:, :], p_y[:, :], scalar1=MAGIC, scalar2=MAGIC,
                    op0=A.add, op1=A.subtract)
                fs_y = ypool.tile([128, NO + NE], f32, tag="yfs")
                nc.vector.scalar_tensor_tensor(
                    fs_y[:, :], p_y[:, :], 1.0, rs_y[:, :],
                    op0=A.mult, op1=A.subtract)
                w = ypool.tile([128, NW], f32, tag="w")
                nc.scalar.activation(
                    w[:, 0:NO], fs_y[:, 0:NO], F.Sin, scale=-TWO_PI)
                nc.scalar.activation(
                    w[:, NO + NE:NW], fs_y[:, 0:NO], F.Sin, scale=TWO_PI)
                sh_y = ypool.tile([128, NE], f32, tag="ysh")
                nc.scalar.activation(
                    sh_y[:, :], fs_y[:, NO:NO + NE], F.Sin, scale=PI)
                sq_y = ypool.tile([128, NE], f32, tag="ysq")
                nc.scalar.activation(sq_y[:, :], sh_y[:, :], F.Square)
                nc.vector.tensor_scalar(
                    w[:, NO:NO + NE], sq_y[:, :], scalar1=-2.0, scalar2=1.0,
                    op0=A.mult, op1=A.add)

                for b in range(B):
                    # --- stage 1: AB_re=[B_odd|A_even], AB_im=[B_even|A_odd]
                    ab_re = psAB.tile([128, NO + NE], f32, tag="ab_re")
                    ab_im = psAB.tile([128, NE + NO], f32, tag="ab_im")
                    # Two accumulation chains share each PSUM bank (one 2KB
                    # zero region): chain 2's start=True must not land between
                    # chain 1's start and its accumulating second matmul, so
                    # order the chains explicitly.
                    chains = [
                        (ab_re[:, 0:NO], sxT, img_odd),
                        (ab_re[:, NO:NO + NE], cxT, img_even),
                        (ab_im[:, 0:NE], sxT, img_even),
                        (ab_im[:, NE:NE + NO], cxT, img_odd),
                    ]
                    prev_last = None
                    for out_ap, tab, img in chains:
                        first = last = None
                        for k in range(2):
                            mm = nc.tensor.matmul(
                                out_ap,
                                tab[k][:, t * 128:(t + 1) * 128],
                                img[(b, k)][:, :],
                                start=(k == 0), stop=(k == 1))
                            if k == 0:
                                first = mm
                            last = mm
                        if prev_last is not None:
                            add_dep_helper(
                                first.ins, prev_last.ins, sync=False,
                                reason="psum zero-region chain ordering")
                        prev_last = last
                    # --- stage 2: Re = sum(AB_re*W[0:257]); -Im = sum(AB_im*W[128:385])
                    scr_re = wpool.tile([128, NO + NE], f32, tag="scr_re")
                    scr_im = wpool.tile([128, NO + NE], f32, tag="scr_im")
                    nc.vector.scalar_tensor_tensor(
                        scr_re[:, :], ab_re[:, :], 1.0, w[:, 0:NO + NE],
                        op0=A.mult, op1=A.mult,
                        accum_out=out_sb[:, (2 * b) * NT + t:(2 * b) * NT + t + 1])
                    nc.vector.scalar_tensor_tensor(
                        scr_im[:, :], ab_im[:, :], 1.0, w[:, NO:NW],
                        op0=A.mult, op1=A.mult,
                        accum_out=out_sb[:, (2 * b + 1) * NT + t:(2 * b + 1) * NT + t + 1])

            nc.sync.dma_start(out[:, :], out_sb[:, :])
            psAB_cm.__exit__(None, None, None)

    nc.compile()
    return nc


def kernel(image, trajectory):
    from concourse.bass_utils import run_bass_kernel_spmd

    if 'nc' not in _CACHE:
        _CACHE['nc'] = _build()
    nc = _CACHE['nc']

    image = np.ascontiguousarray(np.asarray(image, dtype=np.float32))
    trajectory = np.ascontiguousarray(np.asarray(trajectory, dtype=np.float32))
    xs2pi, ysb, ident16 = _consts()

    in_maps = []
    for c in range(NCORES):
        in_maps.append({
            "image": image,
            "traj": np.ascontiguousarray(trajectory[:, c * ML:(c + 1) * ML]),
            "xs2pi": xs2pi,
            "ysb": ysb,
            "ident16": ident16,
        })

    res = run_bass_kernel_spmd(nc, in_maps, core_ids=list(range(NCORES)))

    kspace = np.empty((B, M), dtype=np.complex64)
    for c in range(NCORES):
        o = res.results[c]["out"]          # [128, 4*NT]
        o = o.reshape(128, 2, 2, NT)       # [p, b, (re, -im), t]
        for b in range(B):
            re = o[:, b, 0, :].T.reshape(ML)   # m = t*128 + p
            im = -o[:, b, 1, :].T.reshape(ML)
            kspace[b, c * ML:(c + 1) * ML] = re + 1j * im
    return kspace
